# revision 12
# baseline (speedup 1.0000x reference)
"""Trainium2 Bass kernel for nn_Attention_89670327206161.

Dense transformer attention block, B=8 S=4096 D=1024 H=16 (dh=64), fp32.
The reference contracts attention scores over the *sequence* axis:
    scores_h = K_h^T Q_h / sqrt(dh)   -> (dh, dh) per head
    P_h      = softmax(scores_h, axis=-1)
    out_h    = V_h @ P_h              -> (S, dh)
    out      = concat_h(out_h) @ Wo^T

Sharding: pure data parallelism over batch -- one batch element per
NeuronCore, no collectives. Per core everything streams through SBUF;
only x (transposed on host), the four transposed weights, and the output
touch HBM.

Matmuls run in float32r (full-rate fp32 on the PE at N>=256).

Phase A: stream 32 seq-tiles of 128; compute Q,K (seq-major) with
  PSUM-accumulated projections, immediately accumulate head-pair score
  blocks (two heads packed -> 128x128) into a persistent PSUM tile.
Softmax: exp(0.125*x) per head block (max-subtraction skipped: |logit|
  <~45, safely inside fp32 exp range), row-normalize into a
  block-diagonal P tile per pair.
Phase B: stream 8 seq-blocks of 512; compute V^T (feature-major),
  attention out Z^T = blockdiag(P)^T @ V^T per pair, then the output
  projection back to seq-major, DMA to HBM.
"""

import numpy as np

HEADS = 16
B, S, D = 8, 4096, 1024
DH = D // HEADS          # 64
NPAIR = HEADS // 2       # 8 head pairs -> 128-wide blocks
P = 128                  # partitions
NKC = D // P             # 8 contraction chunks of 128
NT_A = S // P            # 32 seq tiles in phase A
S_BLK = 512
NT_B = S // S_BLK        # 8 seq blocks in phase B
N_CORES = 8

_PROGRAM = None


def _ts(i, n):
    return slice(i * n, (i + 1) * n)


def _build_program():
    import concourse.bacc as bacc
    import concourse.mybir as mybir
    import concourse.tile as tile

    f32 = mybir.dt.float32
    f32r = mybir.dt.float32r
    EXP = mybir.ActivationFunctionType.Exp
    X = mybir.AxisListType.X

    nc = bacc.Bacc(trn_type="TRN2", debug=False, num_devices=N_CORES)

    xT = nc.dram_tensor("xT", [D, S], f32r, kind="ExternalInput")
    wqT = nc.dram_tensor("wqT", [D, D], f32r, kind="ExternalInput")
    wkT = nc.dram_tensor("wkT", [D, D], f32r, kind="ExternalInput")
    wvT = nc.dram_tensor("wvT", [D, D], f32r, kind="ExternalInput")
    woT = nc.dram_tensor("woT", [D, D], f32r, kind="ExternalInput")
    out = nc.dram_tensor("out", [S, D], f32, kind="ExternalOutput")

    xTr = xT.ap().rearrange("(c p) s -> p c s", p=P)      # (128, 8, 4096)

    with tile.TileContext(nc) as tc:
      with tc.tile_pool(name="persist", bufs=1) as persist_pool:
        p_all_pool = persist_pool
        # V/O weights live for the whole kernel; their DMAs overlap phase A.
        wv_sb = persist_pool.tile([P, NKC, D], f32r, tag="wv")
        wo_sb = persist_pool.tile([P, NKC, D], f32r, tag="wo")
        nc.sync.dma_start(wv_sb[:], wvT.ap().rearrange("(c p) o -> p c o", p=P))
        nc.sync.dma_start(wo_sb[:], woT.ap().rearrange("(c p) o -> p c o", p=P))
        with (
            tc.tile_pool(name="w_qk", bufs=1) as w_qk_pool,
            tc.tile_pool(name="const", bufs=1) as const_pool,
            tc.tile_pool(name="sc_ps", bufs=1, space="PSUM") as sc_ps_pool,
        ):
            wq_sb = w_qk_pool.tile([P, NKC, D], f32r, tag="wq")
            wk_sb = w_qk_pool.tile([P, NKC, D], f32r, tag="wk")
            nc.sync.dma_start(wq_sb[:], wqT.ap().rearrange("(c p) o -> p c o", p=P))
            nc.sync.dma_start(wk_sb[:], wkT.ap().rearrange("(c p) o -> p c o", p=P))

            zero_sb = const_pool.tile([P, P], f32r)
            nc.vector.memset(zero_sb[:].bitcast(f32), 0.0)

            # All 8 head-pair score blocks live in one 4-bank PSUM tile for
            # the whole of phase A. Each pair's matmul uses a 256-wide rhs
            # (two pairs of Q columns) so float32r streams at full rate
            # (N>=256); the non-matching 128-col half of each output block is
            # garbage that is never read. Dummy start=True matmuls clear the
            # has_written bits bank-wide; every real score matmul then
            # accumulates with start=False (order-independent).
            scores_ps = sc_ps_pool.tile([P, NPAIR * 256], f32)
            for bank in range(4):
                nc.tensor.matmul(
                    scores_ps[:, _ts(bank, 512)],
                    zero_sb[:],
                    wq_sb[:, 0, 0:512],
                    start=True, stop=False, skip_group_check=True,
                )

            with (
                tc.tile_pool(name="xa", bufs=2) as xa_pool,
                tc.tile_pool(name="qk_sb", bufs=2) as qk_sb_pool,
                tc.tile_pool(name="qk_ps", bufs=3, space="PSUM") as qk_ps_pool,
            ):
                for it in range(NT_A):
                    xa = xa_pool.tile([P, NKC, P], f32r, tag="xa")
                    nc.sync.dma_start(xa[:], xTr[:, :, _ts(it, P)])

                    q_sb = qk_sb_pool.tile([P, D], f32r, tag="q")
                    k_sb = qk_sb_pool.tile([P, D], f32r, tag="k")
                    for w_sb, dst, on_act in ((wq_sb, q_sb, True), (wk_sb, k_sb, False)):
                        for oh in range(2):
                            ps = qk_ps_pool.tile([P, 512], f32, tag="qkps")
                            for ic in range(NKC):
                                nc.tensor.matmul(
                                    ps[:], xa[:, ic, :], w_sb[:, ic, _ts(oh, 512)],
                                    start=(ic == 0), stop=(ic == NKC - 1),
                                )
                            if on_act:
                                nc.scalar.copy(dst[:, _ts(oh, 512)], ps[:])
                            else:
                                nc.vector.tensor_copy(dst[:, _ts(oh, 512)], ps[:])

                    for pr in range(NPAIR):
                        nc.tensor.matmul(
                            scores_ps[:, _ts(pr, 256)],
                            k_sb[:, _ts(pr, P)], q_sb[:, _ts(pr // 2, 256)],
                            start=False, stop=False, skip_group_check=True,
                        )

            # ---- softmax over the q axis of each (dh x dh) head block ----
            p_all = p_all_pool.tile([P, NPAIR, P], f32r)
            nc.vector.memset(p_all[:].bitcast(f32), 0.0)
            with tc.tile_pool(name="smx", bufs=2) as smx_pool:
                for pr in range(NPAIR):
                    # wanted half of the 256-wide output block for pair pr
                    base = pr * 256 + (pr % 2) * P
                    for hf in range(2):
                        rows = slice(64 * hf, 64 * hf + 64)
                        cols = slice(base + 64 * hf, base + 64 * hf + 64)
                        pcols = slice(64 * hf, 64 * hf + 64)
                        # logits reach |142| on real data -- max-subtraction
                        # is required to keep exp inside fp32 range
                        mx = smx_pool.tile([P, 1], f32, tag="mx")
                        nmx = smx_pool.tile([P, 1], f32, tag="nmx")
                        nc.vector.reduce_max(
                            mx[rows, 0:1], scores_ps[rows, cols], axis=X, negate=True
                        )
                        nc.vector.tensor_scalar_mul(nmx[rows, 0:1], mx[rows, 0:1], 0.125)
                        p_tmp = smx_pool.tile([P, 64], f32, tag="ptmp")
                        nc.scalar.activation(
                            p_tmp[rows, :], scores_ps[rows, cols], EXP,
                            bias=nmx[rows, 0:1], scale=0.125,
                        )
                        den = smx_pool.tile([P, 1], f32, tag="den")
                        rec = smx_pool.tile([P, 1], f32, tag="rec")
                        nc.vector.reduce_sum(den[rows, 0:1], p_tmp[rows, :], axis=X)
                        nc.vector.reciprocal(rec[rows, 0:1], den[rows, 0:1])
                        nc.vector.tensor_scalar_mul(
                            p_all[rows, pr, pcols], p_tmp[rows, :], rec[rows, 0:1]
                        )

        # ---- phase B: V^T, attention out, output projection ----
        with (
            tc.tile_pool(name="xb", bufs=2) as xb_pool,
            tc.tile_pool(name="vt", bufs=2) as vt_pool,
            tc.tile_pool(name="zt", bufs=2) as zt_pool,
            tc.tile_pool(name="ob", bufs=2) as ob_pool,
            tc.tile_pool(name="b_ps", bufs=4, space="PSUM") as b_ps_pool,
        ):
            for ib in range(NT_B):
                xb = xb_pool.tile([P, NKC, S_BLK], f32r, tag="xb")
                nc.sync.dma_start(xb[:], xTr[:, :, _ts(ib, S_BLK)])

                vt_sb = vt_pool.tile([P, NKC, S_BLK], f32r, tag="vt")
                for oc in range(NKC):
                    ps = b_ps_pool.tile([P, S_BLK], f32, tag="bps")
                    for ic in range(NKC):
                        nc.tensor.matmul(
                            ps[:], wv_sb[:, ic, _ts(oc, P)], xb[:, ic, :],
                            start=(ic == 0), stop=(ic == NKC - 1),
                        )
                    if oc % 2 == 0:
                        nc.scalar.copy(vt_sb[:, oc, :], ps[:])
                    else:
                        nc.vector.tensor_copy(vt_sb[:, oc, :], ps[:])

                zt_sb = zt_pool.tile([P, NKC, S_BLK], f32r, tag="zt")
                for pr in range(NPAIR):
                    ps = b_ps_pool.tile([P, S_BLK], f32, tag="bps")
                    nc.tensor.matmul(
                        ps[:], p_all[:, pr, :], vt_sb[:, pr, :],
                        start=True, stop=True,
                    )
                    if pr % 2 == 0:
                        nc.vector.tensor_copy(zt_sb[:, pr, :], ps[:])
                    else:
                        nc.scalar.copy(zt_sb[:, pr, :], ps[:])

                for st in range(S_BLK // P):
                    o_sb = ob_pool.tile([P, D], f32, tag="ob")
                    for ot in range(2):
                        ps = b_ps_pool.tile([P, 512], f32, tag="bps")
                        for jc in range(NKC):
                            nc.tensor.matmul(
                                ps[:], zt_sb[:, jc, _ts(st, P)],
                                wo_sb[:, jc, _ts(ot, 512)],
                                start=(jc == 0), stop=(jc == NKC - 1),
                            )
                        if ot == 0:
                            nc.scalar.copy(o_sb[:, _ts(ot, 512)], ps[:])
                        else:
                            nc.vector.tensor_copy(o_sb[:, _ts(ot, 512)], ps[:])
                    r0 = ib * S_BLK + st * P
                    nc.sync.dma_start(out.ap()[r0:r0 + P, :], o_sb[:])

    nc.compile()
    return nc


def _get_program():
    global _PROGRAM
    if _PROGRAM is None:
        _PROGRAM = _build_program()
    return _PROGRAM


def kernel(x, Wq, Wk, Wv, Wo):
    from concourse import bass_utils

    nc = _get_program()

    xT_all = np.ascontiguousarray(np.transpose(np.asarray(x, np.float32), (0, 2, 1)))
    wqT = np.ascontiguousarray(np.asarray(Wq, np.float32).T)
    wkT = np.ascontiguousarray(np.asarray(Wk, np.float32).T)
    wvT = np.ascontiguousarray(np.asarray(Wv, np.float32).T)
    woT = np.ascontiguousarray(np.asarray(Wo, np.float32).T)

    in_maps = [
        {"xT": xT_all[b], "wqT": wqT, "wkT": wkT, "wvT": wvT, "woT": woT}
        for b in range(N_CORES)
    ]
    res = bass_utils.run_bass_kernel_spmd(nc, in_maps, core_ids=list(range(N_CORES)))
    return np.stack([res.results[b]["out"] for b in range(N_CORES)], axis=0)


# revision 23
# speedup vs baseline: 1.4931x; 1.4931x over previous
"""Trainium2 Bass kernel for nn_Attention_89670327206161.

Dense transformer attention block, B=8 S=4096 D=1024 H=16 (dh=64), fp32.
The reference contracts attention scores over the *sequence* axis:
    scores_h = K_h^T Q_h / sqrt(dh)   -> (dh, dh) per head
    P_h      = softmax(scores_h, axis=-1)
    out_h    = V_h @ P_h              -> (S, dh)
    out      = concat_h(out_h) @ Wo^T

Sharding: pure data parallelism over batch -- one batch element per
NeuronCore, no collectives. Per core everything streams through SBUF;
only x (transposed on host), the four transposed weights, and the output
touch HBM.

Matmuls run in float32r (full-rate fp32 on the PE at N>=256).

Phase A: stream 32 seq-tiles of 128; compute Q,K (seq-major) with
  PSUM-accumulated projections, immediately accumulate head-pair score
  blocks (two heads packed -> 128x128) into a persistent PSUM tile.
Softmax: exp(0.125*x) per head block (max-subtraction skipped: |logit|
  <~45, safely inside fp32 exp range), row-normalize into a
  block-diagonal P tile per pair.
Phase B: stream 8 seq-blocks of 512; compute V^T (feature-major),
  attention out Z^T = blockdiag(P)^T @ V^T per pair, then the output
  projection back to seq-major, DMA to HBM.
"""

import numpy as np

HEADS = 16
B, S, D = 8, 4096, 1024
DH = D // HEADS          # 64
NPAIR = HEADS // 2       # 8 head pairs -> 128-wide blocks
P = 128                  # partitions
NKC = D // P             # 8 contraction chunks of 128
NT_A = S // P            # 32 seq tiles in phase A
S_BLK = 512
NT_B = S // S_BLK        # 8 seq blocks in phase B
N_CORES = 8

_PROGRAM = None


def _ts(i, n):
    return slice(i * n, (i + 1) * n)


def _build_program():
    import concourse.bacc as bacc
    import concourse.mybir as mybir
    import concourse.tile as tile

    f32 = mybir.dt.float32
    f32r = mybir.dt.float32r
    EXP = mybir.ActivationFunctionType.Exp
    X = mybir.AxisListType.X

    nc = bacc.Bacc(trn_type="TRN2", debug=False, num_devices=N_CORES)

    xT = nc.dram_tensor("xT", [D, S], f32r, kind="ExternalInput")
    wqT = nc.dram_tensor("wqT", [D, D], f32r, kind="ExternalInput")
    wkT = nc.dram_tensor("wkT", [D, D], f32r, kind="ExternalInput")
    wvT = nc.dram_tensor("wvT", [D, D], f32r, kind="ExternalInput")
    woT = nc.dram_tensor("woT", [D, D], f32r, kind="ExternalInput")
    out = nc.dram_tensor("out", [S, D], f32, kind="ExternalOutput")

    xTr = xT.ap().rearrange("(c p) s -> p c s", p=P)      # (128, 8, 4096)

    with tile.TileContext(nc) as tc:
      with (
          tc.tile_pool(name="persist", bufs=1) as persist_pool,
          # scores PSUM stays reserved through phase B so b_ps lands in the
          # released qk banks instead -- lets the first V^T matmuls overlap
          # the softmax (which is still reading the score banks)
          tc.tile_pool(name="sc_ps", bufs=1, space="PSUM") as sc_ps_pool,
          tc.tile_pool(name="xb", bufs=2) as xb_pool,
      ):
        p_all_pool = persist_pool
        wv_sb = persist_pool.tile([P, NKC, D], f32r, tag="wv")
        wo_sb = persist_pool.tile([P, NKC, D], f32r, tag="wo")
        with (
            tc.tile_pool(name="w_qk", bufs=1) as w_qk_pool,
            tc.tile_pool(name="const", bufs=1) as const_pool,
        ):
            wq_sb = w_qk_pool.tile([P, NKC, D], f32r, tag="wq")
            wk_sb = w_qk_pool.tile([P, NKC, D], f32r, tag="wk")
            # phase-A-critical weights first in the DMA queues; V/O weights
            # queued after (they are only needed in phase B)
            # issue in PE consumption order (Q oh0, Q oh1, K oh0, K oh1) so
            # the first s-tile's matmul groups unblock progressively
            wqTr = wqT.ap().rearrange("(c p) o -> p c o", p=P)
            wkTr = wkT.ap().rearrange("(c p) o -> p c o", p=P)
            nc.sync.dma_start(wq_sb[:, :, 0:512], wqTr[:, :, 0:512])

            zero_sb = const_pool.tile([P, P], f32r)
            nc.vector.memset(zero_sb[:].bitcast(f32), 0.0)

            # All 8 head-pair score blocks live in one 4-bank PSUM tile for
            # the whole of phase A. Each pair's matmul uses a 256-wide rhs
            # (two pairs of Q columns) so float32r streams at full rate
            # (N>=256); the non-matching 128-col half of each output block is
            # garbage that is never read. Dummy start=True matmuls clear the
            # has_written bits bank-wide; every real score matmul then
            # accumulates with start=False (order-independent).
            scores_ps = sc_ps_pool.tile([P, NPAIR * 256], f32)
            for bank in range(4):
                nc.tensor.matmul(
                    scores_ps[:, _ts(bank, 512)],
                    zero_sb[:],
                    wq_sb[:, 0, 0:512],
                    start=True, stop=False, skip_group_check=True,
                )

            with (
                tc.tile_pool(name="xa", bufs=2) as xa_pool,
                tc.tile_pool(name="qk_sb", bufs=2) as qk_sb_pool,
                tc.tile_pool(name="qk_ps", bufs=3, space="PSUM") as qk_ps_pool,
            ):
                # prefetch the first two x slabs and the remaining weight
                # halves in the order PE will consume them; phase-B weights
                # (wv/wo) queue last
                xa_pre = []
                xa = xa_pool.tile([P, NKC, P], f32r, tag="xa")
                nc.sync.dma_start(xa[:], xTr[:, :, _ts(0, P)])
                xa_pre.append(xa)
                nc.sync.dma_start(wq_sb[:, :, 512:D], wqTr[:, :, 512:D])
                nc.sync.dma_start(wk_sb[:, :, 0:512], wkTr[:, :, 0:512])
                nc.sync.dma_start(wk_sb[:, :, 512:D], wkTr[:, :, 512:D])
                xa = xa_pool.tile([P, NKC, P], f32r, tag="xa")
                nc.sync.dma_start(xa[:], xTr[:, :, _ts(1, P)])
                xa_pre.append(xa)
                # background transfers go through SWDGE so they don't sit
                # ahead of the critical xa stream in the HWDGE FIFOs
                nc.gpsimd.dma_start(wv_sb[:], wvT.ap().rearrange("(c p) o -> p c o", p=P))
                nc.gpsimd.dma_start(wo_sb[:], woT.ap().rearrange("(c p) o -> p c o", p=P))
                # prefetch the first phase-B x slab so V^T starts immediately
                xb_pre = xb_pool.tile([P, NKC, S_BLK], f32r, tag="xb")
                nc.gpsimd.dma_start(xb_pre[:], xTr[:, :, _ts(0, S_BLK)])

                for it in range(NT_A):
                    if it < 2:
                        xa = xa_pre[it]
                    else:
                        xa = xa_pool.tile([P, NKC, P], f32r, tag="xa")
                        nc.sync.dma_start(xa[:], xTr[:, :, _ts(it, P)])

                    q_sb = qk_sb_pool.tile([P, D], f32r, tag="q")
                    k_sb = qk_sb_pool.tile([P, D], f32r, tag="k")
                    for w_sb, dst, on_act in ((wq_sb, q_sb, True), (wk_sb, k_sb, False)):
                        for oh in range(2):
                            ps = qk_ps_pool.tile([P, 512], f32, tag="qkps")
                            for ic in range(NKC):
                                nc.tensor.matmul(
                                    ps[:], xa[:, ic, :], w_sb[:, ic, _ts(oh, 512)],
                                    start=(ic == 0), stop=(ic == NKC - 1),
                                )
                            if on_act:
                                nc.scalar.copy(dst[:, _ts(oh, 512)], ps[:])
                            else:
                                nc.vector.tensor_copy(dst[:, _ts(oh, 512)], ps[:])

                    for pr in range(NPAIR):
                        nc.tensor.matmul(
                            scores_ps[:, _ts(pr, 256)],
                            k_sb[:, _ts(pr, P)], q_sb[:, _ts(pr // 2, 256)],
                            start=False, stop=False, skip_group_check=True,
                        )

            # ---- softmax over the q axis of each (dh x dh) head block ----
            p_all = p_all_pool.tile([P, NPAIR, P], f32r)
            nc.vector.memset(p_all[:].bitcast(f32), 0.0)
            with tc.tile_pool(name="smx", bufs=4) as smx_pool:
                for pr in range(NPAIR):
                    # wanted half of the 256-wide output block for pair pr
                    base = pr * 256 + (pr % 2) * P
                    for hf in range(2):
                        rows = slice(64 * hf, 64 * hf + 64)
                        cols = slice(base + 64 * hf, base + 64 * hf + 64)
                        pcols = slice(64 * hf, 64 * hf + 64)
                        # logits reach |142| on real data -- max-subtraction
                        # is required to keep exp inside fp32 range
                        mx = smx_pool.tile([P, 1], f32, tag="mx")
                        nmx = smx_pool.tile([P, 1], f32, tag="nmx")
                        nc.vector.reduce_max(
                            mx[rows, 0:1], scores_ps[rows, cols], axis=X, negate=True
                        )
                        nc.vector.tensor_scalar_mul(nmx[rows, 0:1], mx[rows, 0:1], 0.125)
                        p_tmp = smx_pool.tile([P, 64], f32, tag="ptmp")
                        nc.scalar.activation(
                            p_tmp[rows, :], scores_ps[rows, cols], EXP,
                            bias=nmx[rows, 0:1], scale=0.125,
                        )
                        den = smx_pool.tile([P, 1], f32, tag="den")
                        rec = smx_pool.tile([P, 1], f32, tag="rec")
                        nc.vector.reduce_sum(den[rows, 0:1], p_tmp[rows, :], axis=X)
                        nc.vector.reciprocal(rec[rows, 0:1], den[rows, 0:1])
                        nc.vector.tensor_scalar_mul(
                            p_all[rows, pr, pcols], p_tmp[rows, :], rec[rows, 0:1]
                        )

        # ---- phase B: V^T, attention out, output projection ----
        with (
            tc.tile_pool(name="vt", bufs=2) as vt_pool,
            tc.tile_pool(name="zt", bufs=2) as zt_pool,
            tc.tile_pool(name="ob", bufs=2) as ob_pool,
            tc.tile_pool(name="b_ps", bufs=4, space="PSUM") as b_ps_pool,
        ):
            for ib in range(NT_B):
                if ib == 0:
                    xb = xb_pre
                else:
                    xb = xb_pool.tile([P, NKC, S_BLK], f32r, tag="xb")
                    nc.sync.dma_start(xb[:], xTr[:, :, _ts(ib, S_BLK)])

                vt_sb = vt_pool.tile([P, NKC, S_BLK], f32r, tag="vt")
                for oc in range(NKC):
                    ps = b_ps_pool.tile([P, S_BLK], f32, tag="bps")
                    for ic in range(NKC):
                        nc.tensor.matmul(
                            ps[:], wv_sb[:, ic, _ts(oc, P)], xb[:, ic, :],
                            start=(ic == 0), stop=(ic == NKC - 1),
                        )
                    if oc % 2 == 0:
                        nc.scalar.copy(vt_sb[:, oc, :], ps[:])
                    else:
                        nc.vector.tensor_copy(vt_sb[:, oc, :], ps[:])

                zt_sb = zt_pool.tile([P, NKC, S_BLK], f32r, tag="zt")
                for pr in range(NPAIR):
                    ps = b_ps_pool.tile([P, S_BLK], f32, tag="bps")
                    nc.tensor.matmul(
                        ps[:], p_all[:, pr, :], vt_sb[:, pr, :],
                        start=True, stop=True,
                    )
                    if pr % 2 == 0:
                        nc.vector.tensor_copy(zt_sb[:, pr, :], ps[:])
                    else:
                        nc.scalar.copy(zt_sb[:, pr, :], ps[:])

                for st in range(S_BLK // P):
                    o_sb = ob_pool.tile([P, D], f32, tag="ob")
                    for ot in range(2):
                        ps = b_ps_pool.tile([P, 512], f32, tag="bps")
                        for jc in range(NKC):
                            nc.tensor.matmul(
                                ps[:], zt_sb[:, jc, _ts(st, P)],
                                wo_sb[:, jc, _ts(ot, 512)],
                                start=(jc == 0), stop=(jc == NKC - 1),
                            )
                        if ot == 0:
                            nc.scalar.copy(o_sb[:, _ts(ot, 512)], ps[:])
                        else:
                            nc.vector.tensor_copy(o_sb[:, _ts(ot, 512)], ps[:])
                    r0 = ib * S_BLK + st * P
                    nc.sync.dma_start(out.ap()[r0:r0 + P, :], o_sb[:])

    nc.compile()
    return nc


def _get_program():
    global _PROGRAM
    if _PROGRAM is None:
        _PROGRAM = _build_program()
    return _PROGRAM


def kernel(x, Wq, Wk, Wv, Wo):
    from concourse import bass_utils

    nc = _get_program()

    xT_all = np.ascontiguousarray(np.transpose(np.asarray(x, np.float32), (0, 2, 1)))
    wqT = np.ascontiguousarray(np.asarray(Wq, np.float32).T)
    wkT = np.ascontiguousarray(np.asarray(Wk, np.float32).T)
    wvT = np.ascontiguousarray(np.asarray(Wv, np.float32).T)
    woT = np.ascontiguousarray(np.asarray(Wo, np.float32).T)

    in_maps = [
        {"xT": xT_all[b], "wqT": wqT, "wkT": wkT, "wvT": wvT, "woT": woT}
        for b in range(N_CORES)
    ]
    res = bass_utils.run_bass_kernel_spmd(nc, in_maps, core_ids=list(range(N_CORES)))
    return np.stack([res.results[b]["out"] for b in range(N_CORES)], axis=0)


# revision 31
# speedup vs baseline: 2.5129x; 1.6830x over previous
"""Trainium2 Bass kernel for nn_Attention_89670327206161.

Dense transformer attention block, B=8 S=4096 D=1024 H=16 (dh=64), fp32.
The reference contracts attention scores over the *sequence* axis:
    scores_h = K_h^T Q_h / sqrt(dh)   -> (dh, dh) per head
    P_h      = softmax(scores_h, axis=-1)
    out_h    = V_h @ P_h              -> (S, dh)
    out      = concat_h(out_h) @ Wo^T

Sharding: pure data parallelism over batch -- one batch element per
NeuronCore, no collectives. Per core everything streams through SBUF;
only x (transposed on host), the four transposed weights, and the output
touch HBM.

Matmuls run in float32r (full-rate fp32 on the PE at N>=256).

Phase A: stream 32 seq-tiles of 128; compute Q,K (seq-major) with
  PSUM-accumulated projections, immediately accumulate head-pair score
  blocks (two heads packed -> 128x128) into a persistent PSUM tile.
Softmax: max-subtracted exp(0.125*(x - max)) per head block (logits
  reach |142| on real data, so max-subtraction is mandatory),
  row-normalized into a block-diagonal P tile per pair.
Phase B: stream 8 seq-blocks of 512; compute V^T (feature-major),
  attention out Z^T = blockdiag(P)^T @ V^T per pair, then the output
  projection back to seq-major, DMA to HBM.
"""

import numpy as np

HEADS = 16
B, S, D = 8, 4096, 1024
DH = D // HEADS          # 64
NPAIR = HEADS // 2       # 8 head pairs -> 128-wide blocks
P = 128                  # partitions
NKC = D // P             # 8 contraction chunks of 128
NT_A = S // P            # 32 seq tiles in phase A
S_BLK = 512
NT_B = S // S_BLK        # 8 seq blocks in phase B
N_CORES = 8

_PROGRAM = None


def _ts(i, n):
    return slice(i * n, (i + 1) * n)


def _build_program():
    import concourse.bacc as bacc
    import concourse.mybir as mybir
    import concourse.tile as tile

    f32 = mybir.dt.float32
    f32r = mybir.dt.float32r
    EXP = mybir.ActivationFunctionType.Exp
    X = mybir.AxisListType.X

    nc = bacc.Bacc(trn_type="TRN2", debug=False, num_devices=N_CORES)

    xT = nc.dram_tensor("xT", [D, S], f32r, kind="ExternalInput")
    wqT = nc.dram_tensor("wqT", [D, D], f32r, kind="ExternalInput")
    wkT = nc.dram_tensor("wkT", [D, D], f32r, kind="ExternalInput")
    wvT = nc.dram_tensor("wvT", [D, D], f32r, kind="ExternalInput")
    woT = nc.dram_tensor("woT", [D, D], f32r, kind="ExternalInput")
    out = nc.dram_tensor("out", [S, D], f32, kind="ExternalOutput")

    xTr = xT.ap().rearrange("(c p) s -> p c s", p=P)      # (128, 8, 4096)

    with tile.TileContext(nc) as tc:
      with (
          tc.tile_pool(name="persist", bufs=1) as persist_pool,
          # scores PSUM stays reserved through phase B so b_ps lands in the
          # released qk banks instead -- lets the first V^T matmuls overlap
          # the softmax (which is still reading the score banks)
          tc.tile_pool(name="sc_ps", bufs=1, space="PSUM") as sc_ps_pool,
          tc.tile_pool(name="xb", bufs=2) as xb_pool,
      ):
        p_all_pool = persist_pool
        wv_sb = persist_pool.tile([P, NKC, D], f32r, tag="wv")
        wo_sb = persist_pool.tile([P, NKC, D], f32r, tag="wo")
        with (
            tc.tile_pool(name="w_qk", bufs=1) as w_qk_pool,
            tc.tile_pool(name="const", bufs=1) as const_pool,
        ):
            wq_sb = w_qk_pool.tile([P, NKC, D], f32r, tag="wq")
            wk_sb = w_qk_pool.tile([P, NKC, D], f32r, tag="wk")
            # phase-A-critical weights first in the DMA queues; V/O weights
            # queued after (they are only needed in phase B)
            # issue in PE consumption order (Q oh0, Q oh1, K oh0, K oh1) so
            # the first s-tile's matmul groups unblock progressively
            wqTr = wqT.ap().rearrange("(c p) o -> p c o", p=P)
            wkTr = wkT.ap().rearrange("(c p) o -> p c o", p=P)
            nc.sync.dma_start(wq_sb[:, :, 0:512], wqTr[:, :, 0:512])

            zero_sb = const_pool.tile([P, P], f32r)
            nc.vector.memset(zero_sb[:].bitcast(f32), 0.0)

            # All 8 head-pair score blocks live in one 4-bank PSUM tile for
            # the whole of phase A. Each pair's matmul uses a 256-wide rhs
            # (two pairs of Q columns) so float32r streams at full rate
            # (N>=256); the non-matching 128-col half of each output block is
            # garbage that is never read. Dummy start=True matmuls clear the
            # has_written bits bank-wide; every real score matmul then
            # accumulates with start=False (order-independent).
            scores_ps = sc_ps_pool.tile([P, NPAIR * 256], f32)
            for bank in range(4):
                nc.tensor.matmul(
                    scores_ps[:, _ts(bank, 512)],
                    zero_sb[:],
                    wq_sb[:, 0, 0:512],
                    start=True, stop=False, skip_group_check=True,
                )

            with (
                tc.tile_pool(name="xa", bufs=3) as xa_pool,
                tc.tile_pool(name="qk_sb", bufs=2) as qk_sb_pool,
                tc.tile_pool(name="qk_ps", bufs=4, space="PSUM") as qk_ps_pool,
            ):
                # prefetch the first two x slabs and the remaining weight
                # halves in the order PE will consume them; phase-B weights
                # (wv/wo) queue last
                xa_pre = []
                xa = xa_pool.tile([P, NKC, P], f32r, tag="xa")
                nc.sync.dma_start(xa[:], xTr[:, :, _ts(0, P)])
                xa_pre.append(xa)
                nc.sync.dma_start(wk_sb[:, :, 0:512], wkTr[:, :, 0:512])
                nc.sync.dma_start(wq_sb[:, :, 512:D], wqTr[:, :, 512:D])
                nc.sync.dma_start(wk_sb[:, :, 512:D], wkTr[:, :, 512:D])
                xa = xa_pool.tile([P, NKC, P], f32r, tag="xa")
                nc.sync.dma_start(xa[:], xTr[:, :, _ts(1, P)])
                xa_pre.append(xa)
                # background transfers go through SWDGE so they don't sit
                # ahead of the critical xa stream in the HWDGE FIFOs
                nc.gpsimd.dma_start(wv_sb[:], wvT.ap().rearrange("(c p) o -> p c o", p=P))
                nc.gpsimd.dma_start(wo_sb[:], woT.ap().rearrange("(c p) o -> p c o", p=P))
                # prefetch the first phase-B x slab so V^T starts immediately
                xb_pre = xb_pool.tile([P, NKC, S_BLK], f32r, tag="xb")
                nc.gpsimd.dma_start(xb_pre[:], xTr[:, :, _ts(0, S_BLK)])

                for it in range(NT_A):
                    if it < 2:
                        xa = xa_pre[it]
                    else:
                        xa = xa_pool.tile([P, NKC, P], f32r, tag="xa")
                        nc.sync.dma_start(xa[:], xTr[:, :, _ts(it, P)])

                    q_sb = qk_sb_pool.tile([P, D], f32r, tag="q")
                    k_sb = qk_sb_pool.tile([P, D], f32r, tag="k")
                    if it == 0:
                        # match the weight-DMA arrival order during the ramp
                        groups = [(wq_sb, q_sb, True, 0), (wk_sb, k_sb, False, 0),
                                  (wq_sb, q_sb, True, 1), (wk_sb, k_sb, False, 1)]
                    else:
                        groups = [(wq_sb, q_sb, True, 0), (wq_sb, q_sb, True, 1),
                                  (wk_sb, k_sb, False, 0), (wk_sb, k_sb, False, 1)]
                    for w_sb, dst, on_act, oh in groups:
                        ps = qk_ps_pool.tile([P, 512], f32, tag="qkps")
                        for ic in range(NKC):
                            nc.tensor.matmul(
                                ps[:], xa[:, ic, :], w_sb[:, ic, _ts(oh, 512)],
                                start=(ic == 0), stop=(ic == NKC - 1),
                            )
                        if on_act:
                            nc.scalar.copy(dst[:, _ts(oh, 512)], ps[:])
                        else:
                            nc.vector.tensor_copy(dst[:, _ts(oh, 512)], ps[:])

                    for pr in range(NPAIR):
                        nc.tensor.matmul(
                            scores_ps[:, _ts(pr, 256)],
                            k_sb[:, _ts(pr, P)], q_sb[:, _ts(pr // 2, 256)],
                            start=False, stop=False, skip_group_check=True,
                        )

            # ---- softmax over the q axis of each (dh x dh) head block ----
            p_all = p_all_pool.tile([P, NPAIR, P], f32r)
            nc.vector.memset(p_all[:].bitcast(f32), 0.0)
            with tc.tile_pool(name="smx", bufs=4) as smx_pool:
                for pr in range(NPAIR):
                    # wanted half of the 256-wide output block for pair pr
                    base = pr * 256 + (pr % 2) * P
                    for hf in range(2):
                        rows = slice(64 * hf, 64 * hf + 64)
                        cols = slice(base + 64 * hf, base + 64 * hf + 64)
                        pcols = slice(64 * hf, 64 * hf + 64)
                        # logits reach |142| on real data -- max-subtraction
                        # is required to keep exp inside fp32 range
                        mx = smx_pool.tile([P, 1], f32, tag="mx")
                        nmx = smx_pool.tile([P, 1], f32, tag="nmx")
                        nc.vector.reduce_max(
                            mx[rows, 0:1], scores_ps[rows, cols], axis=X, negate=True
                        )
                        nc.vector.tensor_scalar_mul(nmx[rows, 0:1], mx[rows, 0:1], 0.125)
                        p_tmp = smx_pool.tile([P, 64], f32, tag="ptmp")
                        nc.scalar.activation(
                            p_tmp[rows, :], scores_ps[rows, cols], EXP,
                            bias=nmx[rows, 0:1], scale=0.125,
                        )
                        den = smx_pool.tile([P, 1], f32, tag="den")
                        rec = smx_pool.tile([P, 1], f32, tag="rec")
                        nc.vector.reduce_sum(den[rows, 0:1], p_tmp[rows, :], axis=X)
                        nc.vector.reciprocal(rec[rows, 0:1], den[rows, 0:1])
                        nc.vector.tensor_scalar_mul(
                            p_all[rows, pr, pcols], p_tmp[rows, :], rec[rows, 0:1]
                        )

        # ---- phase B: V^T, attention out, output projection ----
        with (
            tc.tile_pool(name="vt", bufs=2) as vt_pool,
            tc.tile_pool(name="zt", bufs=2) as zt_pool,
            tc.tile_pool(name="ob", bufs=2) as ob_pool,
            tc.tile_pool(name="b_ps", bufs=4, space="PSUM") as b_ps_pool,
        ):
            for ib in range(NT_B):
                if ib == 0:
                    xb = xb_pre
                else:
                    xb = xb_pool.tile([P, NKC, S_BLK], f32r, tag="xb")
                    nc.sync.dma_start(xb[:], xTr[:, :, _ts(ib, S_BLK)])

                vt_sb = vt_pool.tile([P, NKC, S_BLK], f32r, tag="vt")
                for oc in range(NKC):
                    ps = b_ps_pool.tile([P, S_BLK], f32, tag="bps")
                    for ic in range(NKC):
                        nc.tensor.matmul(
                            ps[:], wv_sb[:, ic, _ts(oc, P)], xb[:, ic, :],
                            start=(ic == 0), stop=(ic == NKC - 1),
                        )
                    if oc % 2 == 0:
                        nc.scalar.copy(vt_sb[:, oc, :], ps[:])
                    else:
                        nc.vector.tensor_copy(vt_sb[:, oc, :], ps[:])

                zt_sb = zt_pool.tile([P, NKC, S_BLK], f32r, tag="zt")
                for pr in range(NPAIR):
                    ps = b_ps_pool.tile([P, S_BLK], f32, tag="bps")
                    nc.tensor.matmul(
                        ps[:], p_all[:, pr, :], vt_sb[:, pr, :],
                        start=True, stop=True,
                    )
                    if pr % 2 == 0:
                        nc.vector.tensor_copy(zt_sb[:, pr, :], ps[:])
                    else:
                        nc.scalar.copy(zt_sb[:, pr, :], ps[:])

                for st in range(S_BLK // P):
                    o_sb = ob_pool.tile([P, D], f32, tag="ob")
                    for ot in range(2):
                        ps = b_ps_pool.tile([P, 512], f32, tag="bps")
                        for jc in range(NKC):
                            nc.tensor.matmul(
                                ps[:], zt_sb[:, jc, _ts(st, P)],
                                wo_sb[:, jc, _ts(ot, 512)],
                                start=(jc == 0), stop=(jc == NKC - 1),
                            )
                        if ot == 0:
                            nc.scalar.copy(o_sb[:, _ts(ot, 512)], ps[:])
                        else:
                            nc.vector.tensor_copy(o_sb[:, _ts(ot, 512)], ps[:])
                    r0 = ib * S_BLK + st * P
                    nc.sync.dma_start(out.ap()[r0:r0 + P, :], o_sb[:])

    nc.compile()
    return nc


def _get_program():
    global _PROGRAM
    if _PROGRAM is None:
        _PROGRAM = _build_program()
    return _PROGRAM


def kernel(x, Wq, Wk, Wv, Wo):
    from concourse import bass_utils

    nc = _get_program()

    xT_all = np.ascontiguousarray(np.transpose(np.asarray(x, np.float32), (0, 2, 1)))
    wqT = np.ascontiguousarray(np.asarray(Wq, np.float32).T)
    wkT = np.ascontiguousarray(np.asarray(Wk, np.float32).T)
    wvT = np.ascontiguousarray(np.asarray(Wv, np.float32).T)
    woT = np.ascontiguousarray(np.asarray(Wo, np.float32).T)

    in_maps = [
        {"xT": xT_all[b], "wqT": wqT, "wkT": wkT, "wvT": wvT, "woT": woT}
        for b in range(N_CORES)
    ]
    res = bass_utils.run_bass_kernel_spmd(nc, in_maps, core_ids=list(range(N_CORES)))
    return np.stack([res.results[b]["out"] for b in range(N_CORES)], axis=0)


# revision 33
# speedup vs baseline: 5.9528x; 2.3689x over previous
"""Trainium2 Bass kernel for nn_Attention_89670327206161.

Dense transformer attention block, B=8 S=4096 D=1024 H=16 (dh=64), fp32.
The reference contracts attention scores over the *sequence* axis:
    scores_h = K_h^T Q_h / sqrt(dh)   -> (dh, dh) per head
    P_h      = softmax(scores_h, axis=-1)
    out_h    = V_h @ P_h              -> (S, dh)
    out      = concat_h(out_h) @ Wo^T

Sharding: pure data parallelism over batch -- one batch element per
NeuronCore, no collectives. Per core everything streams through SBUF;
only x (transposed on host), the four transposed weights, and the output
touch HBM.

Matmuls run in float32r (full-rate fp32 on the PE at N>=256).

Phase A: stream 32 seq-tiles of 128; compute Q,K (seq-major) with
  PSUM-accumulated projections, immediately accumulate head-pair score
  blocks (two heads packed -> 128x128) into a persistent PSUM tile.
Softmax: max-subtracted exp(0.125*(x - max)) per head block (logits
  reach |142| on real data, so max-subtraction is mandatory),
  row-normalized into a block-diagonal P tile per pair.
Phase B: stream 8 seq-blocks of 512; compute V^T (feature-major),
  attention out Z^T = blockdiag(P)^T @ V^T per pair, then the output
  projection back to seq-major, DMA to HBM.
"""

import numpy as np

HEADS = 16
B, S, D = 8, 4096, 1024
DH = D // HEADS          # 64
NPAIR = HEADS // 2       # 8 head pairs -> 128-wide blocks
P = 128                  # partitions
NKC = D // P             # 8 contraction chunks of 128
NT_A = S // P            # 32 seq tiles in phase A
S_BLK = 512
NT_B = S // S_BLK        # 8 seq blocks in phase B
N_CORES = 8

_PROGRAM = None


def _ts(i, n):
    return slice(i * n, (i + 1) * n)


def _build_program():
    import concourse.bacc as bacc
    import concourse.mybir as mybir
    import concourse.tile as tile

    f32 = mybir.dt.float32
    f32r = mybir.dt.float32r
    EXP = mybir.ActivationFunctionType.Exp
    X = mybir.AxisListType.X

    nc = bacc.Bacc(trn_type="TRN2", debug=False, num_devices=N_CORES)

    xT = nc.dram_tensor("xT", [D, S], f32r, kind="ExternalInput")
    wqT = nc.dram_tensor("wqT", [D, D], f32r, kind="ExternalInput")
    wkT = nc.dram_tensor("wkT", [D, D], f32r, kind="ExternalInput")
    wvT = nc.dram_tensor("wvT", [D, D], f32r, kind="ExternalInput")
    woT = nc.dram_tensor("woT", [D, D], f32r, kind="ExternalInput")
    out = nc.dram_tensor("out", [S, D], f32, kind="ExternalOutput")

    xTr = xT.ap().rearrange("(c p) s -> p c s", p=P)      # (128, 8, 4096)

    with tile.TileContext(nc) as tc:
      with (
          tc.tile_pool(name="persist", bufs=1) as persist_pool,
          # scores PSUM stays reserved through phase B so b_ps lands in the
          # released qk banks instead -- lets the first V^T matmuls overlap
          # the softmax (which is still reading the score banks)
          tc.tile_pool(name="sc_ps", bufs=1, space="PSUM") as sc_ps_pool,
          tc.tile_pool(name="xb", bufs=2) as xb_pool,
      ):
        p_all_pool = persist_pool
        wv_sb = persist_pool.tile([P, NKC, D], f32r, tag="wv")
        wo_sb = persist_pool.tile([P, NKC, D], f32r, tag="wo")
        with (
            tc.tile_pool(name="w_qk", bufs=1) as w_qk_pool,
            tc.tile_pool(name="const", bufs=1) as const_pool,
        ):
            wq_sb = w_qk_pool.tile([P, NKC, D], f32r, tag="wq")
            wk_sb = w_qk_pool.tile([P, NKC, D], f32r, tag="wk")
            # phase-A-critical weights first in the DMA queues; V/O weights
            # queued after (they are only needed in phase B)
            # issue in PE consumption order (Q oh0, Q oh1, K oh0, K oh1) so
            # the first s-tile's matmul groups unblock progressively
            wqTr = wqT.ap().rearrange("(c p) o -> p c o", p=P)
            wkTr = wkT.ap().rearrange("(c p) o -> p c o", p=P)
            nc.sync.dma_start(wq_sb[:, :, 0:512], wqTr[:, :, 0:512])

            zero_sb = const_pool.tile([P, P], f32r)
            nc.vector.memset(zero_sb[:].bitcast(f32), 0.0)

            # All 8 head-pair score blocks live in one 4-bank PSUM tile for
            # the whole of phase A. Each pair's matmul uses a 256-wide rhs
            # (two pairs of Q columns) so float32r streams at full rate
            # (N>=256); the non-matching 128-col half of each output block is
            # garbage that is never read. Dummy start=True matmuls clear the
            # has_written bits bank-wide; every real score matmul then
            # accumulates with start=False (order-independent).
            scores_ps = sc_ps_pool.tile([P, NPAIR * 256], f32)
            for bank in range(4):
                nc.tensor.matmul(
                    scores_ps[:, _ts(bank, 512)],
                    zero_sb[:],
                    wq_sb[:, 0, 0:512],
                    start=True, stop=False, skip_group_check=True,
                )

            with (
                tc.tile_pool(name="xa", bufs=3) as xa_pool,
                tc.tile_pool(name="qk_sb", bufs=2) as qk_sb_pool,
                tc.tile_pool(name="qk_ps", bufs=4, space="PSUM") as qk_ps_pool,
            ):
                # prefetch the first two x slabs and the remaining weight
                # halves in the order PE will consume them; phase-B weights
                # (wv/wo) queue last
                xa_pre = []
                xa = xa_pool.tile([P, NKC, P], f32r, tag="xa")
                nc.sync.dma_start(xa[:], xTr[:, :, _ts(0, P)])
                xa_pre.append(xa)
                nc.sync.dma_start(wk_sb[:, :, 0:512], wkTr[:, :, 0:512])
                nc.sync.dma_start(wq_sb[:, :, 512:D], wqTr[:, :, 512:D])
                nc.sync.dma_start(wk_sb[:, :, 512:D], wkTr[:, :, 512:D])
                xa = xa_pool.tile([P, NKC, P], f32r, tag="xa")
                nc.sync.dma_start(xa[:], xTr[:, :, _ts(1, P)])
                xa_pre.append(xa)
                # background transfers go through SWDGE so they don't sit
                # ahead of the critical xa stream in the HWDGE FIFOs
                nc.gpsimd.dma_start(wv_sb[:], wvT.ap().rearrange("(c p) o -> p c o", p=P))
                nc.gpsimd.dma_start(wo_sb[:], woT.ap().rearrange("(c p) o -> p c o", p=P))
                # prefetch the first phase-B x slab so V^T starts immediately
                xb_pre = xb_pool.tile([P, NKC, S_BLK], f32r, tag="xb")
                nc.gpsimd.dma_start(xb_pre[:], xTr[:, :, _ts(0, S_BLK)])

                for it in range(NT_A):
                    if it < 2:
                        xa = xa_pre[it]
                    else:
                        xa = xa_pool.tile([P, NKC, P], f32r, tag="xa")
                        nc.sync.dma_start(xa[:], xTr[:, :, _ts(it, P)])

                    q_sb = qk_sb_pool.tile([P, D], f32r, tag="q")
                    k_sb = qk_sb_pool.tile([P, D], f32r, tag="k")
                    if it == 0:
                        # match the weight-DMA arrival order during the ramp
                        groups = [(wq_sb, q_sb, True, 0), (wk_sb, k_sb, False, 0),
                                  (wq_sb, q_sb, True, 1), (wk_sb, k_sb, False, 1)]
                    else:
                        groups = [(wq_sb, q_sb, True, 0), (wq_sb, q_sb, True, 1),
                                  (wk_sb, k_sb, False, 0), (wk_sb, k_sb, False, 1)]
                    for w_sb, dst, on_act, oh in groups:
                        ps = qk_ps_pool.tile([P, 512], f32, tag="qkps")
                        for ic in range(NKC):
                            nc.tensor.matmul(
                                ps[:], xa[:, ic, :], w_sb[:, ic, _ts(oh, 512)],
                                start=(ic == 0), stop=(ic == NKC - 1),
                            )
                        if on_act:
                            nc.scalar.copy(dst[:, _ts(oh, 512)], ps[:])
                        else:
                            nc.vector.tensor_copy(dst[:, _ts(oh, 512)], ps[:])

                    for pr in range(NPAIR):
                        nc.tensor.matmul(
                            scores_ps[:, _ts(pr, 256)],
                            k_sb[:, _ts(pr, P)], q_sb[:, _ts(pr // 2, 256)],
                            start=False, stop=False, skip_group_check=True,
                        )

            # ---- softmax over the q axis of each (dh x dh) head block ----
            p_all = p_all_pool.tile([P, NPAIR, P], f32r)
            nc.vector.memset(p_all[:].bitcast(f32), 0.0)
            with tc.tile_pool(name="smx", bufs=4) as smx_pool:
                for pr in range(NPAIR):
                    # wanted half of the 256-wide output block for pair pr
                    base = pr * 256 + (pr % 2) * P
                    for hf in range(2):
                        rows = slice(64 * hf, 64 * hf + 64)
                        cols = slice(base + 64 * hf, base + 64 * hf + 64)
                        pcols = slice(64 * hf, 64 * hf + 64)
                        # logits reach |142| on real data -- max-subtraction
                        # is required to keep exp inside fp32 range
                        mx = smx_pool.tile([P, 1], f32, tag="mx")
                        nmx = smx_pool.tile([P, 1], f32, tag="nmx")
                        nc.vector.reduce_max(
                            mx[rows, 0:1], scores_ps[rows, cols], axis=X, negate=True
                        )
                        nc.vector.tensor_scalar_mul(nmx[rows, 0:1], mx[rows, 0:1], 0.125)
                        p_tmp = smx_pool.tile([P, 64], f32, tag="ptmp")
                        nc.scalar.activation(
                            p_tmp[rows, :], scores_ps[rows, cols], EXP,
                            bias=nmx[rows, 0:1], scale=0.125,
                        )
                        den = smx_pool.tile([P, 1], f32, tag="den")
                        rec = smx_pool.tile([P, 1], f32, tag="rec")
                        nc.vector.reduce_sum(den[rows, 0:1], p_tmp[rows, :], axis=X)
                        nc.vector.reciprocal(rec[rows, 0:1], den[rows, 0:1])
                        nc.vector.tensor_scalar_mul(
                            p_all[rows, pr, pcols], p_tmp[rows, :], rec[rows, 0:1]
                        )

        # ---- phase B: V^T, attention out, output projection ----
        with (
            tc.tile_pool(name="vt", bufs=2) as vt_pool,
            tc.tile_pool(name="zt", bufs=2) as zt_pool,
            tc.tile_pool(name="ob", bufs=2) as ob_pool,
            tc.tile_pool(name="b_ps", bufs=4, space="PSUM") as b_ps_pool,
        ):
            for ib in range(NT_B):
                if ib == 0:
                    xb = xb_pre
                else:
                    xb = xb_pool.tile([P, NKC, S_BLK], f32r, tag="xb")
                    nc.sync.dma_start(xb[:], xTr[:, :, _ts(ib, S_BLK)])

                vt_sb = vt_pool.tile([P, NKC, S_BLK], f32r, tag="vt")
                for oc in range(NKC):
                    ps = b_ps_pool.tile([P, S_BLK], f32, tag="bps")
                    for ic in range(NKC):
                        nc.tensor.matmul(
                            ps[:], wv_sb[:, ic, _ts(oc, P)], xb[:, ic, :],
                            start=(ic == 0), stop=(ic == NKC - 1),
                        )
                    if oc % 2 == 0:
                        nc.scalar.copy(vt_sb[:, oc, :], ps[:])
                    else:
                        nc.vector.tensor_copy(vt_sb[:, oc, :], ps[:])

                zt_sb = zt_pool.tile([P, NKC, S_BLK], f32r, tag="zt")
                for pr in range(NPAIR):
                    ps = b_ps_pool.tile([P, S_BLK], f32, tag="bps")
                    nc.tensor.matmul(
                        ps[:], p_all[:, pr, :], vt_sb[:, pr, :],
                        start=True, stop=True,
                    )
                    if pr % 2 == 0:
                        nc.vector.tensor_copy(zt_sb[:, pr, :], ps[:])
                    else:
                        nc.scalar.copy(zt_sb[:, pr, :], ps[:])

                for st in range(S_BLK // P):
                    o_sb = ob_pool.tile([P, D], f32, tag="ob")
                    for ot in range(2):
                        ps = b_ps_pool.tile([P, 512], f32, tag="bps")
                        for jc in range(NKC):
                            nc.tensor.matmul(
                                ps[:], zt_sb[:, jc, _ts(st, P)],
                                wo_sb[:, jc, _ts(ot, 512)],
                                start=(jc == 0), stop=(jc == NKC - 1),
                            )
                        if ot == 0:
                            nc.scalar.copy(o_sb[:, _ts(ot, 512)], ps[:])
                        else:
                            nc.vector.tensor_copy(o_sb[:, _ts(ot, 512)], ps[:])
                    r0 = ib * S_BLK + st * P
                    nc.sync.dma_start(out.ap()[r0:r0 + P, :], o_sb[:])

    nc.compile()
    return nc


def _get_program():
    global _PROGRAM
    if _PROGRAM is None:
        _PROGRAM = _build_program()
    return _PROGRAM


def kernel(x, Wq, Wk, Wv, Wo):
    from concourse import bass_utils

    nc = _get_program()

    xT_all = np.ascontiguousarray(np.transpose(np.asarray(x, np.float32), (0, 2, 1)))
    wqT = np.ascontiguousarray(np.asarray(Wq, np.float32).T)
    wkT = np.ascontiguousarray(np.asarray(Wk, np.float32).T)
    wvT = np.ascontiguousarray(np.asarray(Wv, np.float32).T)
    woT = np.ascontiguousarray(np.asarray(Wo, np.float32).T)

    in_maps = [
        {"xT": xT_all[b], "wqT": wqT, "wkT": wkT, "wvT": wvT, "woT": woT}
        for b in range(N_CORES)
    ]
    res = bass_utils.run_bass_kernel_spmd(nc, in_maps, core_ids=list(range(N_CORES)))
    return np.stack([res.results[b]["out"] for b in range(N_CORES)], axis=0)
